# revision 10
# baseline (speedup 1.0000x reference)
"""Trainium2 Bass kernel for AutoRegressiveGraphConvLayer.

Contract: kernel(**inputs) takes the FULL unsharded inputs (as produced by
setup_inputs()) and returns (output_nodes [8,1024,64], output_edges [8,16248,32]).
Data-parallel over batch: core b computes batch element b.

Structure exploited (static, from the problem definition):
  Edge e enumerates (j, i) pairs with i in [1, 1024), j in [max(0, i-16), i),
  sorted by i then j. Pad edges of node i into 16 slots s (right-aligned:
  j = i - 16 + s), giving a token domain t = 16*i + s of size 16384.
  - input_nodes[node_src]  -> sliding-window access pattern over nodes
  - input_nodes[edge_dst]  -> per-node broadcast access pattern
  - node scatter-add       -> per-16-slot segment sum
  - prev-edge scatter-add  -> per-16-slot exclusive prefix scan
"""
import os
import sys

if os.path.isdir("/opt/trn_rl_repo") and "/opt/trn_rl_repo" not in sys.path:
    sys.path.insert(0, "/opt/trn_rl_repo")

import numpy as np

import concourse.bass as bass
import concourse.mybir as mybir
import concourse.tile as tile
from concourse.ap import AP

F32 = mybir.dt.float32
F32R = mybir.dt.float32r
BF16 = mybir.dt.bfloat16
AF = mybir.ActivationFunctionType
ALU = mybir.AluOpType

# Problem sizes (static)
N, M, B = 1024, 16, 8
FN, FE = 64, 32
AN, AE = 128, 128
ON, OE = 64, 32
T = N * M            # 16384 padded tokens
NE = 16248           # real edge count
NT = 32              # 512-token tiles
TILE = 512

# Matmul input mode: "f32r" uses the fast fp32 PE path (1 cyc/row vs 4).
MM_DTYPE = os.environ.get("KERNEL_MM_DTYPE", "f32r")
DT_MM = None  # set in build_module


# ----------------------------------------------------------------------------
# Host-side static index structures
# ----------------------------------------------------------------------------
def _static_maps():
    i = np.arange(N)[:, None]          # node
    s = np.arange(M)[None, :]          # slot
    start = np.maximum(0, M - i)       # first valid slot of node i
    valid = (s >= start) & (i >= 1)    # node 0 has no edges
    pos = s - start                    # edge position within node (valid only)
    cnt = np.minimum(i, M).ravel().astype(np.int64)
    cnt[0] = 0
    base = np.concatenate([[0], np.cumsum(cnt)[:-1]])
    e_of_t = (base[:, None] + pos)     # [N, M], valid entries give edge id
    tok_of_e = np.zeros(NE, np.int64)
    tok = (np.arange(N * M)).reshape(N, M)
    tok_of_e[e_of_t[valid]] = tok[valid]
    # norm over previous edges: 1/pos (pos>0); 1 at pos==0; 0 at invalid
    enorm = np.where(valid & (pos > 0), 1.0 / np.maximum(pos, 1), np.where(valid, 1.0, 0.0))
    return valid, e_of_t, tok_of_e, enorm.astype(np.float32)


_VALID, _E_OF_T, _TOK_OF_E, _ENORM_T = _static_maps()


def _packed_const(values_t, head):
    """Arrange per-token values [T] into the packed scan layout [128, 512]:
    partition p = 32*g + f (f = feature, ignored), col c, token = 512*(4*sb+g)+c.
    head=True -> superblock 0 layout; else the bulk (periodic) pattern taken
    from superblock 1 (all bulk superblocks are identical)."""
    sb = 0 if head else 1
    out = np.zeros((128, TILE), np.float32)
    for g in range(4):
        toks = values_t[TILE * (4 * sb + g): TILE * (4 * sb + g + 1)]
        out[32 * g: 32 * (g + 1), :] = toks[None, :]
    return out


def _host_constants(W1n, b1n, W2n, b2n, Wln, bln, W1e, b1e, W2e, b2e, Wle, ble,
                    norm_node):
    c = {}
    # weights as lhsT tiles (K on partitions)
    c["w1n_src"] = W1n[0:64, :]                                   # [64, 256]
    c["w1n_e"] = np.vstack([W1n[64:96, :], b1n[None, :]])          # [33, 256]
    c["w2n"] = np.concatenate([W2n[0:128, :], W2n[128:256, :]], axis=1)  # [128, 256]
    c["b2n"] = b2n[:, None]                                        # [128, 1]
    c["wln_a"] = Wln[0:128, :]                                     # [128, 64]
    c["wln_b"] = np.vstack([Wln[128:192, :], bln[None, :]])        # [65, 64]
    c["w1e_src"] = W1e[0:64, :]                                    # [64, 256]
    c["w1e_e"] = np.vstack([W1e[64:96, :], b1e[None, :]])          # [33, 256]
    c["w1e_dst"] = W1e[96:160, :]                                  # [64, 256]
    c["w2e"] = np.concatenate([W2e[0:128, :], W2e[128:256, :]], axis=1)  # [128, 256]
    c["b2e"] = b2e[:, None]                                        # [128, 1]
    c["wle_a"] = Wle[0:128, :]                                     # [128, 32]
    c["wle_b"] = np.vstack([Wle[128:160, :], ble[None, :]])        # [33, 32]

    valid_t = _VALID.reshape(-1).astype(np.float32)                # [T]
    # a-path mask for tokens 0..255 (all invalid tokens live there)
    c["amask"] = np.broadcast_to(valid_t[None, 0:256], (128, 256)).copy()
    # scan-input mask: zero at slot 15 and at invalid tokens
    s_of_t = np.arange(T) % M
    xzmask_t = valid_t * (s_of_t != 15)
    c["pmask_h"] = _packed_const(xzmask_t, head=True)
    c["pmask_b"] = _packed_const(xzmask_t, head=False)
    # scan reset mask: zero at slot 0
    rmask_t = (s_of_t != 0).astype(np.float32)
    c["rmask"] = _packed_const(rmask_t, head=False)
    # edge norms in packed layout
    enorm_t = _ENORM_T.reshape(-1)
    c["enorm_h"] = _packed_const(enorm_t, head=True)
    c["enorm_b"] = _packed_const(enorm_t, head=False)
    # node norm in the raggP layout [128, 16, 32]:
    # raggP[64*cc + f, q, j] = node_agg_proj[f, 64*q + 32*cc + j]
    nn = np.zeros((128, 16, 32), np.float32)
    for cc in range(2):
        for q in range(16):
            nn[64 * cc: 64 * (cc + 1), q, :] = norm_node[64 * q + 32 * cc + np.arange(32)][None, :]
    c["nnormP"] = nn.reshape(128, 512)
    return c


def _host_activations(input_nodes_b, input_edges_b):
    """Per-batch-element transposed/padded activations."""
    nodesT = np.zeros((65, 1040), np.float32)
    nodesT[0:64, 16:1040] = input_nodes_b.T
    nodesT[64, :] = 1.0
    eT = np.zeros((33, T), np.float32)
    padded = np.zeros((T, FE), np.float32)
    padded[_TOK_OF_E, :] = input_edges_b
    eT[0:32, :] = padded.T
    eT[32, :] = 1.0
    return nodesT, eT


# ----------------------------------------------------------------------------
# Device module
# ----------------------------------------------------------------------------
MM_CONSTS = {"w1n_src", "w1n_e", "w2n", "wln_b", "w1e_src",
             "w1e_e", "w1e_dst", "w2e"}
BF_CONSTS = {"wln_a", "wle_a", "wle_b"}
CONST_NAMES = [
    ("w1n_src", [64, 256]), ("w1n_e", [33, 256]), ("w2n", [128, 256]),
    ("b2n", [128, 1]), ("wln_a", [128, 64]), ("wln_b", [65, 64]),
    ("w1e_src", [64, 256]), ("w1e_e", [33, 256]), ("w1e_dst", [64, 256]),
    ("w2e", [128, 256]), ("b2e", [128, 1]), ("wle_a", [128, 32]),
    ("wle_b", [33, 32]),
    ("amask", [128, 256]), ("pmask_h", [128, 512]), ("pmask_b", [128, 512]),
    ("rmask", [128, 512]), ("enorm_h", [128, 512]), ("enorm_b", [128, 512]),
    ("nnormP", [128, 512]),
]


def _legalize_multi_waits(nc):
    """The installed walrus supports a single sync-wait per instruction
    (codegen 'Too many sync wait commands'). Hoist all-but-one wait of any
    multi-wait instruction into standalone EventSemaphore waits on the same
    engine, inserted immediately before it (same engine stream => identical
    blocking semantics)."""
    fn = nc.m.functions[0]
    for blk in fn.blocks:
        out = []
        changed = False
        for inst in blk.instructions:
            si = inst.sync_info
            waits = list(si.on_wait) if si else []
            if len(waits) > 1 and all(w.wait_reg is None for w in waits):
                changed = True
                for k, w in enumerate(waits[:-1]):
                    ev = mybir.InstEventSemaphore(
                        name=f"{inst.name}-hoistw{k}", ins=[], outs=[])
                    ev.engine = inst.engine
                    ev.sync_info = mybir.SyncInfo(on_wait=[w], on_update=[])
                    nc.inst_map[ev.name] = ev
                    out.append(ev)
                inst.sync_info = mybir.SyncInfo(
                    on_wait=[waits[-1]], on_update=list(si.on_update))
            out.append(inst)
        if changed:
            blk.instructions = out


def build_module():
    nc = bass.Bass()
    use_f32r = (MM_DTYPE == "f32r")
    DTM = F32R if use_f32r else F32
    dram = {}
    dram["nodesT"] = nc.dram_tensor("nodesT", [65, 1040], DTM, kind="ExternalInput")
    dram["eT"] = nc.dram_tensor("eT", [33, T], DTM, kind="ExternalInput")
    for nm, shp in CONST_NAMES:
        dt_ = DTM if nm in MM_CONSTS else (BF16 if nm in BF_CONSTS else F32)
        dram[nm] = nc.dram_tensor(nm, shp, dt_, kind="ExternalInput")
    dram["eT_bf"] = nc.dram_tensor("eT_bf", [33, T], BF16, kind="ExternalInput")
    out_nodesT_d = nc.dram_tensor("out_nodesT", [64, 1024], F32, kind="ExternalOutput")
    out_edgesT_d = nc.dram_tensor("out_edgesT", [32, T], F32, kind="ExternalOutput")

    def mm(out, lhsT, rhs, start, stop, tp=None):
        kw = {}
        if tp is not None:
            kw["tile_position"] = tp
        nc.tensor.matmul(out, lhsT, rhs, start=start, stop=stop, **kw)

    with tile.TileContext(nc) as tc:
        with (
            tc.tile_pool(name="pers", bufs=1) as pers,
            tc.tile_pool(name="yp", bufs=2) as yp,
            tc.tile_pool(name="ap_", bufs=2) as ap_,
            tc.tile_pool(name="sc", bufs=2) as sc,
            tc.tile_pool(name="ps_y", bufs=3, space="PSUM") as ps_y,
            tc.tile_pool(name="ps_z", bufs=2, space="PSUM") as ps_z,
            tc.tile_pool(name="ps_acc", bufs=2, space="PSUM") as ps_acc,
            tc.tile_pool(name="ps_q", bufs=1, space="PSUM") as ps_q,
        ):
            # ---- persistent loads
            sb = {}
            sb["nodesT"] = pers.tile([65, 1040], DTM, tag="nodesT", name="nodesT_sb")
            sb["eT"] = pers.tile([33, T], DTM, tag="eT", name="eT_sb")
            nc.sync.dma_start(sb["nodesT"][:], dram["nodesT"][:])
            for q4 in range(4):
                nc.sync.dma_start(sb["eT"][:, q4 * 4096:(q4 + 1) * 4096],
                                  dram["eT"][:, q4 * 4096:(q4 + 1) * 4096])
            for nm, shp in CONST_NAMES:
                dt_ = DTM if nm in MM_CONSTS else (BF16 if nm in BF_CONSTS else F32)
                sb[nm] = pers.tile(shp, dt_, tag=nm, name=nm + "_sb")
                nc.sync.dma_start(sb[nm][:], dram[nm][:])
            sb["eT_bf"] = pers.tile([33, T], BF16, tag="eT_bf", name="eT_bf_sb")
            for q4 in range(4):
                nc.sync.dma_start(sb["eT_bf"][:, q4 * 4096:(q4 + 1) * 4096],
                                  dram["eT_bf"][:, q4 * 4096:(q4 + 1) * 4096])

            raggP = pers.tile([128, 16, 32], F32, tag="raggP")
            raggPn = pers.tile([128, 16, 32], F32, tag="raggPn")

            # --- wait absorbers -------------------------------------------
            # Matmult codegen supports a single sync-wait; make each engine
            # observe every persistent load's DMA semaphore once (via 1-elem
            # dummy ops) so real instructions never need two waits.
            dummy_ps = ps_q.tile([2, 16], F32, tag="q", name="dummy_ps")
            scratch = pers.tile([1, 16], F32, tag="scratch", name="scratch_sb")
            pe_tiles = ["nodesT", "eT", "eT_bf", "w1n_src", "w1n_e", "w2n",
                        "wln_a", "wln_b", "w1e_src", "w1e_e", "w1e_dst", "w2e",
                        "wle_a", "wle_b"]
            for nm in pe_tiles:
                tl = sb[nm]
                nc.tensor.matmul(dummy_ps[0:2, 0:2], tl[0:1, 0:2], tl[0:1, 0:2],
                                 start=True, stop=True)
            for nm in ("b2n", "b2e"):
                nc.scalar.copy(scratch[0:1, 0:1], sb[nm][0:1, 0:1])
            zconst = nc.const_aps.scalar_like(0.0, sb["b2n"][:])
            nc.scalar.copy(scratch[0:1, 2:3], zconst[0:1, :])
            for nm in ("amask", "pmask_h", "pmask_b", "rmask", "enorm_h",
                       "enorm_b", "nnormP"):
                nc.vector.tensor_copy(scratch[0:1, 1:2], sb[nm][0:1, 0:1])

            nodesT_h = sb["nodesT"][:]

            def xsrc_ap(t):
                # [64 feat, 32 nodes, 16 slots]: col = 32 t + di + s
                return AP(nodesT_h.tensor, nodesT_h.offset + 32 * t,
                          [[1040, 64], [1, 32], [1, 16]])

            def xdst_ap(t):
                return sb["nodesT"][0:64, 16 + 32 * t: 16 + 32 * t + 32] \
                    .unsqueeze(2).to_broadcast([64, 32, 16])

            def relu_drain(dst, src_psum, idx, bias=None):
                if bias is not None:
                    nc.scalar.activation(dst, src_psum, AF.Relu, bias=bias)
                elif idx % 2 == 0:
                    nc.scalar.activation(dst, src_psum, AF.Relu)
                else:
                    nc.vector.tensor_scalar_max(dst, src_psum, 0.0)

            # =========================== node path ===========================
            ps_r = None
            for t in range(NT):
                et_tile = sb["eT"][:, TILE * t: TILE * (t + 1)]
                yt = yp.tile([128, 2, TILE], DTM, tag="yt")
                for cch in range(2):
                    cs = slice(128 * cch, 128 * (cch + 1))
                    py = ps_y.tile([128, TILE], F32, tag="y")
                    mm(py[:], sb["w1n_src"][:, cs], xsrc_ap(t), True, False)
                    mm(py[:], sb["w1n_e"][:, cs], et_tile, False, True)
                    relu_drain(yt[:, cch, :], py[:], 2 * t + cch)
                pz = ps_z.tile([128, TILE], F32, tag="z")
                mm(pz[:], sb["w2n"][:, 0:128], yt[:, 0, :], True, False)
                mm(pz[:], sb["w2n"][:, 128:256], yt[:, 1, :], False, True)
                at = ap_.tile([128, TILE], BF16, tag="at")
                nc.scalar.activation(at[:], pz[:], AF.Relu, bias=sb["b2n"][:])
                if t == 0:
                    nc.vector.tensor_tensor(at[:, 0:256], at[:, 0:256],
                                            sb["amask"][:], ALU.mult)
                pos = t & 1
                if pos == 0:
                    ps_r = ps_acc.tile([128, TILE], F32, tag="acc")
                mm(ps_r[64 * pos: 64 * (pos + 1), :], sb["wln_a"][:], at[:],
                   True, True, tp=(0, 64 * pos))
                if pos == 1:
                    nc.vector.tensor_reduce(
                        raggP[:, t >> 1, :],
                        ps_r[:].rearrange("p (n s) -> p n s", s=16),
                        axis=mybir.AxisListType.X, op=ALU.add)

            # =========================== edge path ===========================
            ps_p = None
            for t in range(NT):
                g, sblk = t & 3, t >> 2
                et_tile = sb["eT"][:, TILE * t: TILE * (t + 1)]
                yt = yp.tile([128, 2, TILE], DTM, tag="yt")
                for cch in range(2):
                    cs = slice(128 * cch, 128 * (cch + 1))
                    py = ps_y.tile([128, TILE], F32, tag="y")
                    mm(py[:], sb["w1e_src"][:, cs], xsrc_ap(t), True, False)
                    mm(py[:], sb["w1e_e"][:, cs], et_tile, False, False)
                    mm(py[:], sb["w1e_dst"][:, cs], xdst_ap(t), False, True)
                    relu_drain(yt[:, cch, :], py[:], 2 * t + cch)
                pz = ps_z.tile([128, TILE], F32, tag="z")
                mm(pz[:], sb["w2e"][:, 0:128], yt[:, 0, :], True, False)
                mm(pz[:], sb["w2e"][:, 128:256], yt[:, 1, :], False, True)
                aet = ap_.tile([128, TILE], BF16, tag="at")
                nc.scalar.activation(aet[:], pz[:], AF.Relu, bias=sb["b2e"][:])
                if g == 0:
                    ps_p = ps_acc.tile([128, TILE], F32, tag="acc")
                mm(ps_p[32 * g: 32 * (g + 1), :], sb["wle_a"][:], aet[:],
                   True, True, tp=(0, 32 * g))
                if g == 3:
                    # Q = Wle_b^T E + ble for the whole superblock, col-tiled
                    pq = ps_q.tile([128, TILE], F32, tag="q")
                    for gg in range(4):
                        tt = 4 * sblk + gg
                        mm(pq[32 * gg: 32 * (gg + 1), :], sb["wle_b"][:],
                           sb["eT_bf"][:, TILE * tt: TILE * (tt + 1)],
                           True, True, tp=(0, 32 * gg))
                    pm = sb["pmask_h"] if sblk == 0 else sb["pmask_b"]
                    en = sb["enorm_h"] if sblk == 0 else sb["enorm_b"]
                    xz = sc.tile([128, 520], F32, tag="xz")
                    nc.vector.memset(xz[:, 0:1], 0.0)
                    nc.vector.tensor_tensor(xz[:, 1:513], ps_p[:], pm[:], ALU.mult)
                    excl = sc.tile([128, TILE], F32, tag="excl")
                    nc.vector.tensor_tensor_scan(
                        excl[:], sb["rmask"][:], xz[:, 0:512], 0.0,
                        ALU.mult, ALU.add)
                    nc.vector.tensor_tensor(excl[:], excl[:], en[:], ALU.mult)
                    res = sc.tile([128, TILE], F32, tag="res")
                    nc.vector.tensor_tensor(res[:], excl[:], pq[:], ALU.add)
                    nc.vector.tensor_scalar_max(res[:], res[:], 0.0)
                    for gg in range(4):
                        tt = 4 * sblk + gg
                        nc.sync.dma_start(
                            out_edgesT_d[:, TILE * tt: TILE * (tt + 1)],
                            res[32 * gg: 32 * (gg + 1), :])

            # =========================== node final ==========================
            nc.vector.tensor_tensor(raggPn[:], raggP[:], sb["nnormP"][:], ALU.mult)
            for h in range(2):
                pw = ps_q.tile([64, TILE], F32, tag="q")
                mm(pw[:], sb["wln_b"][:],
                   sb["nodesT"][0:65, 16 + TILE * h: 16 + TILE * (h + 1)],
                   True, True)
                outn = sc.tile([64, TILE], F32, tag="res")
                pw_r = pw[:].rearrange("p (q c j) -> p q c j", q=8, c=2, j=32)
                outn_r = outn[:].rearrange("p (q c j) -> p q c j", q=8, c=2, j=32)
                for cc in range(2):
                    nc.vector.tensor_tensor(
                        outn_r[:, :, cc, :], pw_r[:, :, cc, :],
                        raggPn[64 * cc: 64 * (cc + 1), 8 * h: 8 * (h + 1), :],
                        ALU.add)
                nc.vector.tensor_scalar_max(outn[:], outn[:], 0.0)
                nc.sync.dma_start(out_nodesT_d[:, TILE * h: TILE * (h + 1)], outn[:])

    _legalize_multi_waits(nc)
    nc.finalize()
    return nc


_MODULE = None


def _get_module():
    global _MODULE
    if _MODULE is None:
        _MODULE = build_module()
    return _MODULE


def make_in_maps(input_nodes, input_edges, W1n, b1n, W2n, b2n, Wln, bln,
                 W1e, b1e, W2e, b2e, Wle, ble, norm_node):
    import ml_dtypes
    consts = _host_constants(W1n, b1n, W2n, b2n, Wln, bln,
                             W1e, b1e, W2e, b2e, Wle, ble, norm_node)
    consts = {k: (np.ascontiguousarray(v, ml_dtypes.bfloat16) if k in BF_CONSTS
                  else np.ascontiguousarray(v, np.float32))
              for k, v in consts.items()}
    in_maps = []
    for b in range(B):
        nodesT, eT = _host_activations(input_nodes[b], input_edges[b])
        im = dict(consts)
        im["nodesT"] = nodesT
        im["eT"] = eT
        im["eT_bf"] = eT.astype(ml_dtypes.bfloat16)
        in_maps.append(im)
    return in_maps


def unpack_outputs(results):
    output_nodes = np.zeros((B, N, FN), np.float32)
    output_edges = np.zeros((B, NE, FE), np.float32)
    for b in range(B):
        output_nodes[b] = results[b]["out_nodesT"].T
        output_edges[b] = results[b]["out_edgesT"][:, _TOK_OF_E].T
    return output_nodes, output_edges


def run(inputs, trace=False):
    from concourse import bass_utils
    nc = _get_module()
    args = {k: np.asarray(v, np.float32) for k, v in inputs.items()
            if k in ("input_nodes", "input_edges", "W1n", "b1n", "W2n", "b2n",
                     "Wln", "bln", "W1e", "b1e", "W2e", "b2e", "Wle", "ble",
                     "norm_node")}
    in_maps = make_in_maps(**args)
    res = bass_utils.run_bass_kernel_spmd(
        nc, in_maps, core_ids=list(range(B)), trace=trace)
    return unpack_outputs(res.results), res


def kernel(**inputs):
    (output_nodes, output_edges), _ = run(inputs, trace=False)
    return output_nodes, output_edges


# revision 11
# speedup vs baseline: 4760.8504x; 4760.8504x over previous
"""Trainium2 Bass kernel for AutoRegressiveGraphConvLayer.

Contract: kernel(**inputs) takes the FULL unsharded inputs (as produced by
setup_inputs()) and returns (output_nodes [8,1024,64], output_edges [8,16248,32]).
Data-parallel over batch: core b computes batch element b.

Structure exploited (static, from the problem definition):
  Edge e enumerates (j, i) pairs with i in [1, 1024), j in [max(0, i-16), i),
  sorted by i then j. Pad edges of node i into 16 slots s (right-aligned:
  j = i - 16 + s), giving a token domain t = 16*i + s of size 16384.
  - input_nodes[node_src]  -> sliding-window access pattern over nodes
  - input_nodes[edge_dst]  -> per-node broadcast access pattern
  - node scatter-add       -> per-16-slot segment sum
  - prev-edge scatter-add  -> per-16-slot exclusive prefix scan
"""
import os
import sys

if os.path.isdir("/opt/trn_rl_repo") and "/opt/trn_rl_repo" not in sys.path:
    sys.path.insert(0, "/opt/trn_rl_repo")

import numpy as np

import concourse.bass as bass
import concourse.mybir as mybir
import concourse.tile as tile
from concourse.ap import AP

F32 = mybir.dt.float32
F32R = mybir.dt.float32r
BF16 = mybir.dt.bfloat16
AF = mybir.ActivationFunctionType
ALU = mybir.AluOpType

# Problem sizes (static)
N, M, B = 1024, 16, 8
FN, FE = 64, 32
AN, AE = 128, 128
ON, OE = 64, 32
T = N * M            # 16384 padded tokens
NE = 16248           # real edge count
NT = 32              # 512-token tiles
TILE = 512

# Matmul input mode: "f32r" uses the fast fp32 PE path (1 cyc/row vs 4).
MM_DTYPE = os.environ.get("KERNEL_MM_DTYPE", "f32r")
PHASES = os.environ.get("KERNEL_PHASES", "all")  # comma set: node,edge,final
DT_MM = None  # set in build_module


# ----------------------------------------------------------------------------
# Host-side static index structures
# ----------------------------------------------------------------------------
def _static_maps():
    i = np.arange(N)[:, None]          # node
    s = np.arange(M)[None, :]          # slot
    start = np.maximum(0, M - i)       # first valid slot of node i
    valid = (s >= start) & (i >= 1)    # node 0 has no edges
    pos = s - start                    # edge position within node (valid only)
    cnt = np.minimum(i, M).ravel().astype(np.int64)
    cnt[0] = 0
    base = np.concatenate([[0], np.cumsum(cnt)[:-1]])
    e_of_t = (base[:, None] + pos)     # [N, M], valid entries give edge id
    tok_of_e = np.zeros(NE, np.int64)
    tok = (np.arange(N * M)).reshape(N, M)
    tok_of_e[e_of_t[valid]] = tok[valid]
    # norm over previous edges: 1/pos (pos>0); 1 at pos==0; 0 at invalid
    enorm = np.where(valid & (pos > 0), 1.0 / np.maximum(pos, 1), np.where(valid, 1.0, 0.0))
    return valid, e_of_t, tok_of_e, enorm.astype(np.float32)


_VALID, _E_OF_T, _TOK_OF_E, _ENORM_T = _static_maps()


def _packed_const(values_t, head):
    """Arrange per-token values [T] into the packed scan layout [128, 512]:
    partition p = 32*g + f (f = feature, ignored), col c, token = 512*(4*sb+g)+c.
    head=True -> superblock 0 layout; else the bulk (periodic) pattern taken
    from superblock 1 (all bulk superblocks are identical)."""
    sb = 0 if head else 1
    out = np.zeros((128, TILE), np.float32)
    for g in range(4):
        toks = values_t[TILE * (4 * sb + g): TILE * (4 * sb + g + 1)]
        out[32 * g: 32 * (g + 1), :] = toks[None, :]
    return out


def _host_constants(W1n, b1n, W2n, b2n, Wln, bln, W1e, b1e, W2e, b2e, Wle, ble,
                    norm_node):
    c = {}
    # weights as lhsT tiles (K on partitions)
    c["w1n_src"] = W1n[0:64, :]                                   # [64, 256]
    c["w1n_e"] = np.vstack([W1n[64:96, :], b1n[None, :]])          # [33, 256]
    c["w2n"] = np.concatenate([W2n[0:128, :], W2n[128:256, :]], axis=1)  # [128, 256]
    c["b2n"] = b2n[:, None]                                        # [128, 1]
    c["wln_a"] = Wln[0:128, :]                                     # [128, 64]
    c["wln_b"] = np.vstack([Wln[128:192, :], bln[None, :]])        # [65, 64]
    c["w1e_src"] = W1e[0:64, :]                                    # [64, 256]
    c["w1e_e"] = np.vstack([W1e[64:96, :], b1e[None, :]])          # [33, 256]
    c["w1e_dst"] = W1e[96:160, :]                                  # [64, 256]
    c["w2e"] = np.concatenate([W2e[0:128, :], W2e[128:256, :]], axis=1)  # [128, 256]
    c["b2e"] = b2e[:, None]                                        # [128, 1]
    c["wle_a"] = Wle[0:128, :]                                     # [128, 32]
    c["wle_b"] = np.vstack([Wle[128:160, :], ble[None, :]])        # [33, 32]

    valid_t = _VALID.reshape(-1).astype(np.float32)                # [T]
    # a-path mask for tokens 0..255 (all invalid tokens live there)
    c["amask"] = np.broadcast_to(valid_t[None, 0:256], (128, 256)).copy()
    # scan-input mask: zero at slot 15 and at invalid tokens
    s_of_t = np.arange(T) % M
    xzmask_t = valid_t * (s_of_t != 15)
    c["pmask_h"] = _packed_const(xzmask_t, head=True)
    c["pmask_b"] = _packed_const(xzmask_t, head=False)
    # scan reset mask: zero at slot 0
    rmask_t = (s_of_t != 0).astype(np.float32)
    c["rmask"] = _packed_const(rmask_t, head=False)
    # edge norms in packed layout
    enorm_t = _ENORM_T.reshape(-1)
    c["enorm_h"] = _packed_const(enorm_t, head=True)
    c["enorm_b"] = _packed_const(enorm_t, head=False)
    # node norm in the raggP layout [128, 16, 32]:
    # raggP[64*cc + f, q, j] = node_agg_proj[f, 64*q + 32*cc + j]
    nn = np.zeros((128, 16, 32), np.float32)
    for cc in range(2):
        for q in range(16):
            nn[64 * cc: 64 * (cc + 1), q, :] = norm_node[64 * q + 32 * cc + np.arange(32)][None, :]
    c["nnormP"] = nn.reshape(128, 512)
    return c


def _host_activations(input_nodes_b, input_edges_b):
    """Per-batch-element transposed/padded activations."""
    nodesT = np.zeros((65, 1040), np.float32)
    nodesT[0:64, 16:1040] = input_nodes_b.T
    nodesT[64, :] = 1.0
    eT = np.zeros((33, T), np.float32)
    padded = np.zeros((T, FE), np.float32)
    padded[_TOK_OF_E, :] = input_edges_b
    eT[0:32, :] = padded.T
    eT[32, :] = 1.0
    return nodesT, eT


# ----------------------------------------------------------------------------
# Device module
# ----------------------------------------------------------------------------
MM_CONSTS = {"w1n_src", "w1n_e", "w2n", "wln_b", "w1e_src",
             "w1e_e", "w1e_dst", "w2e"}
BF_CONSTS = {"wln_a", "wle_a", "wle_b"}
CONST_NAMES = [
    ("w1n_src", [64, 256]), ("w1n_e", [33, 256]), ("w2n", [128, 256]),
    ("b2n", [128, 1]), ("wln_a", [128, 64]), ("wln_b", [65, 64]),
    ("w1e_src", [64, 256]), ("w1e_e", [33, 256]), ("w1e_dst", [64, 256]),
    ("w2e", [128, 256]), ("b2e", [128, 1]), ("wle_a", [128, 32]),
    ("wle_b", [33, 32]),
    ("amask", [128, 256]), ("pmask_h", [128, 512]), ("pmask_b", [128, 512]),
    ("rmask", [128, 512]), ("enorm_h", [128, 512]), ("enorm_b", [128, 512]),
    ("nnormP", [128, 512]),
]


def _legalize_multi_waits(nc):
    """The installed walrus supports a single sync-wait per instruction
    (codegen 'Too many sync wait commands'). Hoist all-but-one wait of any
    multi-wait instruction into standalone EventSemaphore waits on the same
    engine, inserted immediately before it (same engine stream => identical
    blocking semantics)."""
    fn = nc.m.functions[0]
    for blk in fn.blocks:
        out = []
        changed = False
        for inst in blk.instructions:
            si = inst.sync_info
            waits = list(si.on_wait) if si else []
            if len(waits) > 1 and all(w.wait_reg is None for w in waits):
                changed = True
                for k, w in enumerate(waits[:-1]):
                    ev = mybir.InstEventSemaphore(
                        name=f"{inst.name}-hoistw{k}", ins=[], outs=[])
                    ev.engine = inst.engine
                    ev.sync_info = mybir.SyncInfo(on_wait=[w], on_update=[])
                    nc.inst_map[ev.name] = ev
                    out.append(ev)
                inst.sync_info = mybir.SyncInfo(
                    on_wait=[waits[-1]], on_update=list(si.on_update))
            out.append(inst)
        if changed:
            blk.instructions = out


def build_module():
    nc = bass.Bass()
    use_f32r = (MM_DTYPE == "f32r")
    DTM = F32R if use_f32r else F32
    dram = {}
    dram["nodesT"] = nc.dram_tensor("nodesT", [65, 1040], DTM, kind="ExternalInput")
    dram["eT"] = nc.dram_tensor("eT", [33, T], DTM, kind="ExternalInput")
    for nm, shp in CONST_NAMES:
        dt_ = DTM if nm in MM_CONSTS else (BF16 if nm in BF_CONSTS else F32)
        dram[nm] = nc.dram_tensor(nm, shp, dt_, kind="ExternalInput")
    dram["eT_bf"] = nc.dram_tensor("eT_bf", [33, T], BF16, kind="ExternalInput")
    out_nodesT_d = nc.dram_tensor("out_nodesT", [64, 1024], F32, kind="ExternalOutput")
    out_edgesT_d = nc.dram_tensor("out_edgesT", [32, T], F32, kind="ExternalOutput")

    def mm(out, lhsT, rhs, start, stop, tp=None):
        kw = {}
        if tp is not None:
            kw["tile_position"] = tp
        nc.tensor.matmul(out, lhsT, rhs, start=start, stop=stop, **kw)

    with tile.TileContext(nc) as tc:
        with (
            tc.tile_pool(name="pers", bufs=1) as pers,
            tc.tile_pool(name="yp", bufs=2) as yp,
            tc.tile_pool(name="ap_", bufs=2) as ap_,
            tc.tile_pool(name="sc", bufs=2) as sc,
            tc.tile_pool(name="ps_y", bufs=3, space="PSUM") as ps_y,
            tc.tile_pool(name="ps_z", bufs=2, space="PSUM") as ps_z,
            tc.tile_pool(name="ps_acc", bufs=2, space="PSUM") as ps_acc,
            tc.tile_pool(name="ps_q", bufs=1, space="PSUM") as ps_q,
        ):
            # ---- persistent loads
            sb = {}
            sb["nodesT"] = pers.tile([65, 1040], DTM, tag="nodesT", name="nodesT_sb")
            sb["eT"] = pers.tile([33, T], DTM, tag="eT", name="eT_sb")
            nc.sync.dma_start(sb["nodesT"][:], dram["nodesT"][:])
            for q4 in range(4):
                nc.sync.dma_start(sb["eT"][:, q4 * 4096:(q4 + 1) * 4096],
                                  dram["eT"][:, q4 * 4096:(q4 + 1) * 4096])
            for nm, shp in CONST_NAMES:
                dt_ = DTM if nm in MM_CONSTS else (BF16 if nm in BF_CONSTS else F32)
                sb[nm] = pers.tile(shp, dt_, tag=nm, name=nm + "_sb")
                nc.sync.dma_start(sb[nm][:], dram[nm][:])
            sb["eT_bf"] = pers.tile([33, T], BF16, tag="eT_bf", name="eT_bf_sb")
            for q4 in range(4):
                nc.sync.dma_start(sb["eT_bf"][:, q4 * 4096:(q4 + 1) * 4096],
                                  dram["eT_bf"][:, q4 * 4096:(q4 + 1) * 4096])

            raggP = pers.tile([128, 16, 32], F32, tag="raggP")
            raggPn = pers.tile([128, 16, 32], F32, tag="raggPn")

            # --- wait absorbers -------------------------------------------
            # Matmult codegen supports a single sync-wait; make each engine
            # observe every persistent load's DMA semaphore once (via 1-elem
            # dummy ops) so real instructions never need two waits.
            dummy_ps = ps_q.tile([2, 16], F32, tag="q", name="dummy_ps")
            scratch = pers.tile([1, 16], F32, tag="scratch", name="scratch_sb")
            pe_tiles = ["nodesT", "eT", "eT_bf", "w1n_src", "w1n_e", "w2n",
                        "wln_a", "wln_b", "w1e_src", "w1e_e", "w1e_dst", "w2e",
                        "wle_a", "wle_b"]
            for nm in pe_tiles:
                tl = sb[nm]
                nc.tensor.matmul(dummy_ps[0:2, 0:2], tl[0:1, 0:2], tl[0:1, 0:2],
                                 start=True, stop=True)
            for nm in ("b2n", "b2e"):
                nc.scalar.copy(scratch[0:1, 0:1], sb[nm][0:1, 0:1])
            zconst = nc.const_aps.scalar_like(0.0, sb["b2n"][:])
            nc.scalar.copy(scratch[0:1, 2:3], zconst[0:1, :])
            for nm in ("amask", "pmask_h", "pmask_b", "rmask", "enorm_h",
                       "enorm_b", "nnormP"):
                nc.vector.tensor_copy(scratch[0:1, 1:2], sb[nm][0:1, 0:1])

            nodesT_h = sb["nodesT"][:]

            def xsrc_ap(t):
                # [64 feat, 32 nodes, 16 slots]: col = 32 t + di + s
                return AP(nodesT_h.tensor, nodesT_h.offset + 32 * t,
                          [[1040, 64], [1, 32], [1, 16]])

            def xdst_ap(t):
                return sb["nodesT"][0:64, 16 + 32 * t: 16 + 32 * t + 32] \
                    .unsqueeze(2).to_broadcast([64, 32, 16])

            def relu_drain(dst, src_psum, idx, bias=None):
                if bias is not None:
                    nc.scalar.activation(dst, src_psum, AF.Relu, bias=bias)
                elif idx % 2 == 0:
                    nc.scalar.activation(dst, src_psum, AF.Relu)
                else:
                    nc.vector.tensor_scalar_max(dst, src_psum, 0.0)

            # =========================== node path ===========================
            _want = (lambda p: PHASES == "all" or p in PHASES.split(","))
            ps_r = None
            for t in range(NT) if _want("node") else []:
                et_tile = sb["eT"][:, TILE * t: TILE * (t + 1)]
                yt = yp.tile([128, 2, TILE], DTM, tag="yt")
                for cch in range(2):
                    cs = slice(128 * cch, 128 * (cch + 1))
                    py = ps_y.tile([128, TILE], F32, tag="y")
                    mm(py[:], sb["w1n_src"][:, cs], xsrc_ap(t), True, False)
                    mm(py[:], sb["w1n_e"][:, cs], et_tile, False, True)
                    relu_drain(yt[:, cch, :], py[:], 2 * t + cch)
                pz = ps_z.tile([128, TILE], F32, tag="z")
                mm(pz[:], sb["w2n"][:, 0:128], yt[:, 0, :], True, False)
                mm(pz[:], sb["w2n"][:, 128:256], yt[:, 1, :], False, True)
                at = ap_.tile([128, TILE], BF16, tag="at")
                nc.scalar.activation(at[:], pz[:], AF.Relu, bias=sb["b2n"][:])
                if t == 0:
                    nc.vector.tensor_tensor(at[:, 0:256], at[:, 0:256],
                                            sb["amask"][:], ALU.mult)
                pos = t & 1
                if pos == 0:
                    ps_r = ps_acc.tile([128, TILE], F32, tag="acc")
                mm(ps_r[64 * pos: 64 * (pos + 1), :], sb["wln_a"][:], at[:],
                   True, True, tp=(0, 64 * pos))
                if pos == 1:
                    nc.vector.tensor_reduce(
                        raggP[:, t >> 1, :],
                        ps_r[:].rearrange("p (n s) -> p n s", s=16),
                        axis=mybir.AxisListType.X, op=ALU.add)

            # =========================== edge path ===========================
            ps_p = None
            for t in range(NT) if _want("edge") else []:
                g, sblk = t & 3, t >> 2
                et_tile = sb["eT"][:, TILE * t: TILE * (t + 1)]
                yt = yp.tile([128, 2, TILE], DTM, tag="yt")
                for cch in range(2):
                    cs = slice(128 * cch, 128 * (cch + 1))
                    py = ps_y.tile([128, TILE], F32, tag="y")
                    mm(py[:], sb["w1e_src"][:, cs], xsrc_ap(t), True, False)
                    mm(py[:], sb["w1e_e"][:, cs], et_tile, False, False)
                    mm(py[:], sb["w1e_dst"][:, cs], xdst_ap(t), False, True)
                    relu_drain(yt[:, cch, :], py[:], 2 * t + cch)
                pz = ps_z.tile([128, TILE], F32, tag="z")
                mm(pz[:], sb["w2e"][:, 0:128], yt[:, 0, :], True, False)
                mm(pz[:], sb["w2e"][:, 128:256], yt[:, 1, :], False, True)
                aet = ap_.tile([128, TILE], BF16, tag="at")
                nc.scalar.activation(aet[:], pz[:], AF.Relu, bias=sb["b2e"][:])
                if g == 0:
                    ps_p = ps_acc.tile([128, TILE], F32, tag="acc")
                mm(ps_p[32 * g: 32 * (g + 1), :], sb["wle_a"][:], aet[:],
                   True, True, tp=(0, 32 * g))
                if g == 3:
                    # Q = Wle_b^T E + ble for the whole superblock, col-tiled
                    pq = ps_q.tile([128, TILE], F32, tag="q")
                    for gg in range(4):
                        tt = 4 * sblk + gg
                        mm(pq[32 * gg: 32 * (gg + 1), :], sb["wle_b"][:],
                           sb["eT_bf"][:, TILE * tt: TILE * (tt + 1)],
                           True, True, tp=(0, 32 * gg))
                    pm = sb["pmask_h"] if sblk == 0 else sb["pmask_b"]
                    en = sb["enorm_h"] if sblk == 0 else sb["enorm_b"]
                    xz = sc.tile([128, 520], F32, tag="xz")
                    nc.vector.memset(xz[:, 0:1], 0.0)
                    nc.vector.tensor_tensor(xz[:, 1:513], ps_p[:], pm[:], ALU.mult)
                    excl = sc.tile([128, TILE], F32, tag="excl")
                    nc.vector.tensor_tensor_scan(
                        excl[:], sb["rmask"][:], xz[:, 0:512], 0.0,
                        ALU.mult, ALU.add)
                    nc.vector.tensor_tensor(excl[:], excl[:], en[:], ALU.mult)
                    res = sc.tile([128, TILE], F32, tag="res")
                    nc.vector.tensor_tensor(res[:], excl[:], pq[:], ALU.add)
                    nc.vector.tensor_scalar_max(res[:], res[:], 0.0)
                    for gg in range(4):
                        tt = 4 * sblk + gg
                        nc.sync.dma_start(
                            out_edgesT_d[:, TILE * tt: TILE * (tt + 1)],
                            res[32 * gg: 32 * (gg + 1), :])

            # =========================== node final ==========================
            if not _want("node"):
                nc.vector.memset(raggP[:], 0.0)
            nc.vector.tensor_tensor(raggPn[:], raggP[:], sb["nnormP"][:], ALU.mult)
            for h in range(2) if _want("final") else []:
                pw = ps_q.tile([64, TILE], F32, tag="q")
                mm(pw[:], sb["wln_b"][:],
                   sb["nodesT"][0:65, 16 + TILE * h: 16 + TILE * (h + 1)],
                   True, True)
                outn = sc.tile([64, TILE], F32, tag="res")
                pw_r = pw[:].rearrange("p (q c j) -> p q c j", q=8, c=2, j=32)
                outn_r = outn[:].rearrange("p (q c j) -> p q c j", q=8, c=2, j=32)
                for cc in range(2):
                    nc.vector.tensor_tensor(
                        outn_r[:, :, cc, :], pw_r[:, :, cc, :],
                        raggPn[64 * cc: 64 * (cc + 1), 8 * h: 8 * (h + 1), :],
                        ALU.add)
                nc.vector.tensor_scalar_max(outn[:], outn[:], 0.0)
                nc.sync.dma_start(out_nodesT_d[:, TILE * h: TILE * (h + 1)], outn[:])

    _legalize_multi_waits(nc)
    nc.finalize()
    return nc


_MODULE = None


def _get_module():
    global _MODULE
    if _MODULE is None:
        _MODULE = build_module()
    return _MODULE


def make_in_maps(input_nodes, input_edges, W1n, b1n, W2n, b2n, Wln, bln,
                 W1e, b1e, W2e, b2e, Wle, ble, norm_node):
    import ml_dtypes
    consts = _host_constants(W1n, b1n, W2n, b2n, Wln, bln,
                             W1e, b1e, W2e, b2e, Wle, ble, norm_node)
    consts = {k: (np.ascontiguousarray(v, ml_dtypes.bfloat16) if k in BF_CONSTS
                  else np.ascontiguousarray(v, np.float32))
              for k, v in consts.items()}
    in_maps = []
    for b in range(B):
        nodesT, eT = _host_activations(input_nodes[b], input_edges[b])
        im = dict(consts)
        im["nodesT"] = nodesT
        im["eT"] = eT
        im["eT_bf"] = eT.astype(ml_dtypes.bfloat16)
        in_maps.append(im)
    return in_maps


def unpack_outputs(results):
    output_nodes = np.zeros((B, N, FN), np.float32)
    output_edges = np.zeros((B, NE, FE), np.float32)
    for b in range(B):
        output_nodes[b] = results[b]["out_nodesT"].T
        output_edges[b] = results[b]["out_edgesT"][:, _TOK_OF_E].T
    return output_nodes, output_edges


def run(inputs, trace=False):
    from concourse import bass_utils
    nc = _get_module()
    args = {k: np.asarray(v, np.float32) for k, v in inputs.items()
            if k in ("input_nodes", "input_edges", "W1n", "b1n", "W2n", "b2n",
                     "Wln", "bln", "W1e", "b1e", "W2e", "b2e", "Wle", "ble",
                     "norm_node")}
    in_maps = make_in_maps(**args)
    res = bass_utils.run_bass_kernel_spmd(
        nc, in_maps, core_ids=list(range(B)), trace=trace)
    return unpack_outputs(res.results), res


def kernel(**inputs):
    (output_nodes, output_edges), _ = run(inputs, trace=False)
    return output_nodes, output_edges
